# revision 27
# baseline (speedup 1.0000x reference)
"""MoE FFN layer (8 experts, top-2, SwiGLU) on 8 Trainium2 NeuronCores.

Strategy: expert parallelism. The gate (softmax + top-2 + combine weights +
aux loss) is a tiny [T,8] computation done on host as part of the
dispatch/combine (all-to-all) step. Each of the 8 cores runs ONE expert's
SwiGLU FFN over the tokens routed to it (capacity-padded so all cores run
the same SPMD program):

    hT  [2I, C] = w_gu  @ x_e.T          (K = H = 1024)
    aT  [ I, C] = silu(hT[:I]) * hT[I:]
    yT  [ H, C] = w_down @ aT            (K = I = 4096)

All tensors are pre-arranged on host into the exact SBUF layouts the
TensorEngine wants (K on partitions), so the device kernel is pure
matmul + activation with zero on-chip transposes. The weighted
scatter-add combine happens at unshard time on host.

Hardcoded problem shape: x [4, 2048, 1024], E=8, I=4096, top-2.
"""

import numpy as np

H = 1024
I = 4096
E = 8
TOP_K = 2
LB_WEIGHT = 0.01
NT = 512          # token tile (matmul moving-operand free dim)
P = 128

# matmul dtype: float32r = relaxed fp32. On TRN2 it streams at 1 PE
# cycle/row (bf16 speed) when the moving free dim is >= 256, vs 4
# cycles/row for exact fp32 — at ~2.5e-4 relative error end to end.
_MM_DTYPE_NAME = "float32r"

_NC_CACHE = {}


SUPER = 640       # max tokens per weight-streaming block (bounded by SBUF)


def _blocks_for(C):
    """Split C into weight-streaming blocks. Two goals, in order:
    1. Minimize the number of PSUM sub-chunks (each block splits into
       sub-chunks of <=512): matmul instruction count scales with sub-chunk
       count and each instruction carries ~25ns of unmodeled HW overhead.
    2. Keep every sub-chunk >= 256 (float32r runs 1 PE cycle/row only for
       moving free dim >= 256; 4 cycles/row below).
    C is a multiple of 64 and >= 512 with C % 512 not in (128, 256) —
    see the capacity rounding in kernel(). A remainder r <= 128 folds into
    a final (512+r) block (subs [256+r, 256]); r >= 256 is its own block."""
    assert C >= 512 and C % 64 == 0
    a, r = divmod(C, 512)
    assert r == 0 or r <= 128 or r >= 256, f"bad capacity {C}"
    blocks = [512] * a
    if 0 < r <= 128:
        blocks = blocks[:-1] + [512 + r]
    elif r:
        blocks.append(r)
    return blocks


def _subs_for(b):
    if b <= 512:
        return [b]
    return [b - 256, 256]


def _chunks_for(C):
    out = []
    for b in _blocks_for(C):
        out.extend(_subs_for(b))
    return out


def _emit_ffn(tc, nc, mybir, x, w1, wd, y, C):
    """Emit the per-core fused SwiGLU FFN.

    x  : DRAM [128, 8*C]   x[p, ko*C + n] = x_tok[n, ko*128 + p]
    w1 : DRAM [8192, 1024] w1[(pair*2+s)*128 + p, ko*128 + m]
                               = gate_up[s*I + pair*128 + m, ko*128 + p]
    wd : DRAM [1024, 4096] wd[m2*128 + p, k*128 + m] = down[m2*128 + m, k*128 + p]
    y  : DRAM [1024, C]    y[m2*128 + m, n] = out_tok[n, m2*128 + m]
    """
    import concourse.bass as bass  # noqa: F401

    f32 = mybir.dt.float32
    mmdt = getattr(mybir.dt, _MM_DTYPE_NAME)
    sigmoid = mybir.ActivationFunctionType.Sigmoid
    KO = H // P            # 8  k-tiles in stage 1
    NPAIR = I // P         # 32 (g,u) row-pair tiles
    K2 = I // P            # 32 k-tiles in stage 2
    M2 = H // P            # 8  output row tiles

    x3 = x.rearrange("p (ko n) -> p ko n", n=C)
    w13 = w1.rearrange("(pr s p) c -> p pr s c", s=2, p=P)
    wd3h = wd.rearrange("(m2 p) (h k c) -> p m2 h k c", p=P, h=2, c=P)
    y3 = y.rearrange("(m2 p) n -> p m2 n", p=P)

    NPRE = 2               # weight pairs prefetched across the block boundary

    with (
        tc.tile_pool(name="xp", bufs=2) as xp,
        tc.tile_pool(name="w1p", bufs=3) as w1p,
        tc.tile_pool(name="wdp", bufs=4) as wdp,
        tc.tile_pool(name="ap", bufs=1) as apool,
        tc.tile_pool(name="sp", bufs=2) as spool,
        tc.tile_pool(name="s2p", bufs=2) as spool2,
        tc.tile_pool(name="yp", bufs=2) as ypool,
        tc.tile_pool(name="ps", bufs=8, space="PSUM") as psum,
    ):
        blocks = _blocks_for(C)
        starts = [sum(blocks[:i]) for i in range(len(blocks))]

        def load_x(bi):
            xt = xp.tile([P, KO, SUPER], mmdt, tag="x")
            nc.sync.dma_start(
                xt[:, :, :blocks[bi]], x3[:, :, starts[bi]:starts[bi] + blocks[bi]]
            )
            return xt

        def load_wgu(pair):
            # one DMA brings this pair's g row-block and u row-block
            wgu = w1p.tile([P, 2, H], mmdt, tag="w1")
            nc.sync.dma_start(wgu[:], w13[:, pair, :, :])
            return wgu

        xt = load_x(0)
        wgu_pre = [load_wgu(p) for p in range(NPRE)]

        for bi, S in enumerate(blocks):
            n0 = starts[bi]
            subs = []
            o = 0
            for nt in _subs_for(S):
                subs.append((o, nt))
                o += nt

            at = apool.tile([P, K2, SUPER], mmdt, tag="a")

            for pair in range(NPAIR):
                wgu = wgu_pre[pair] if pair < NPRE else load_wgu(pair)
                for (o, nt) in subs:
                    pg = psum.tile([P, nt], f32, tag="ps")
                    pu = psum.tile([P, nt], f32, tag="ps")
                    for ko in range(KO):
                        nc.tensor.matmul(
                            pg[:],
                            wgu[:, 0, ko * P:(ko + 1) * P],
                            xt[:, ko, o:o + nt],
                            start=(ko == 0),
                            stop=(ko == KO - 1),
                        )
                    for ko in range(KO):
                        nc.tensor.matmul(
                            pu[:],
                            wgu[:, 1, ko * P:(ko + 1) * P],
                            xt[:, ko, o:o + nt],
                            start=(ko == 0),
                            stop=(ko == KO - 1),
                        )
                    sg = spool.tile([P, NT], f32, tag="s")
                    nc.scalar.activation(sg[:, :nt], pg[:], sigmoid)
                    sm = spool2.tile([P, NT], f32, tag="s2")
                    nc.vector.tensor_mul(sm[:, :nt], sg[:, :nt], pg[:])
                    nc.vector.tensor_mul(at[:, pair, o:o + nt], sm[:, :nt], pu[:])

            # Prefetch the next block's x and first weight pairs NOW, so those
            # DMAs enter the FIFO sync ring ahead of stage-2's ~10MB of
            # y-writes (else the next block's first matmuls stall ~6us).
            if bi + 1 < len(blocks):
                xt = load_x(bi + 1)
                wgu_pre = [load_wgu(p) for p in range(NPRE)]

            for m2 in range(M2):
                # wd streamed in halves for finer prefetch granularity
                wlo = wdp.tile([P, K2 // 2, P], mmdt, tag="wd")
                nc.sync.dma_start(wlo[:], wd3h[:, m2, 0])
                whi = wdp.tile([P, K2 // 2, P], mmdt, tag="wd")
                nc.sync.dma_start(whi[:], wd3h[:, m2, 1])
                ysb = ypool.tile([P, SUPER], f32, tag="y")
                for (o, nt) in subs:
                    py = psum.tile([P, nt], f32, tag="ps")
                    for k in range(K2):
                        wdt = wlo if k < K2 // 2 else whi
                        nc.tensor.matmul(
                            py[:],
                            wdt[:, k % (K2 // 2), :],
                            at[:, k, o:o + nt],
                            start=(k == 0),
                            stop=(k == K2 - 1),
                        )
                    nc.vector.tensor_copy(ysb[:, o:o + nt], py[:])
                nc.sync.dma_start(y3[:, m2, n0:n0 + S], ysb[:, :S])


def _build_nc(C, repeat=1):
    import concourse.tile as tile
    from concourse import bacc, mybir

    nc = bacc.Bacc("TRN2", target_bir_lowering=False, debug=False)
    f32 = mybir.dt.float32
    mmdt = getattr(mybir.dt, _MM_DTYPE_NAME)
    x = nc.dram_tensor("x", [P, (H // P) * C], mmdt, kind="ExternalInput").ap()
    w1 = nc.dram_tensor("w1", [2 * I, H], mmdt, kind="ExternalInput").ap()
    wd = nc.dram_tensor("wd", [H, I], mmdt, kind="ExternalInput").ap()
    y = nc.dram_tensor("y", [H, C], f32, kind="ExternalOutput").ap()
    with tile.TileContext(nc) as tc:
        for _ in range(repeat):
            _emit_ffn(tc, nc, mybir, x, w1, wd, y, C)
    nc.compile()
    return nc


def _get_nc(C):
    if C not in _NC_CACHE:
        _NC_CACHE[C] = _build_nc(C)
    return _NC_CACHE[C]


def _route(xf, gate_w):
    """Host gating: returns (probs, top2 idx, normalized top2 probs)."""
    logits = xf @ gate_w.T                                   # [T, E] f32
    m = logits.max(axis=-1, keepdims=True)
    ex = np.exp(logits - m)
    probs = ex / ex.sum(axis=-1, keepdims=True)
    order = np.argsort(-probs, axis=1, kind="stable")
    idx = order[:, :TOP_K]                                   # [T, 2]
    tp = np.take_along_axis(probs, idx, axis=1)
    tp = tp / (tp.sum(axis=-1, keepdims=True) + 1e-9)
    return probs, idx, tp


def _pack_expert_weights(gate_up_e, down_e):
    w1h = np.ascontiguousarray(
        gate_up_e.reshape(2, I // P, P, H // P, P).transpose(1, 0, 4, 3, 2)
    ).reshape(2 * I, H)
    wdh = np.ascontiguousarray(
        down_e.reshape(H // P, P, I // P, P).transpose(0, 3, 2, 1)
    ).reshape(H, I)
    return w1h, wdh


_LAST_RUN = None


def benchmark(iters=10, warmup=2, nc=None, in_maps=None):
    """Wall-clock the device execution of the last kernel() call's NEFF.

    Re-drives the same PJRT/shard_map path run_bass_kernel_spmd uses under
    axon, but with device-resident inputs and repeated pipelined calls so
    the per-call time approximates actual device execution time (max over
    the 8 cores). Returns ns per iteration.
    """
    import time
    import jax
    import numpy as np
    from jax.experimental.shard_map import shard_map
    from jax.sharding import Mesh, NamedSharding, PartitionSpec
    from concourse import bass2jax, mybir

    if nc is None:
        assert _LAST_RUN is not None, "call kernel() first"
        nc, in_maps = _LAST_RUN
    n_cores = len(in_maps)
    bass2jax.install_neuronx_cc_hook()

    partition_name = (
        nc.partition_id_tensor.name if nc.partition_id_tensor else None
    )
    in_names, out_names, out_avals, zero_outs = [], [], [], []
    for alloc in nc.m.functions[0].allocations:
        if not isinstance(alloc, bass2jax.mybir.MemoryLocationSet):
            continue
        name = alloc.memorylocations[0].name
        if alloc.kind == "ExternalInput":
            if name != partition_name:
                in_names.append(name)
        elif alloc.kind == "ExternalOutput":
            out_names.append(name)
            shape = tuple(alloc.tensor_shape)
            dtype = mybir.dt.np(alloc.dtype)
            out_avals.append(jax.core.ShapedArray(shape, dtype))
            zero_outs.append(np.zeros(shape, dtype))
    n_params = len(in_names)
    all_names = in_names + out_names
    if partition_name is not None:
        all_names = all_names + [partition_name]

    def _body(*args):
        operands = list(args)
        if partition_name is not None:
            operands.append(bass2jax.partition_id_tensor())
        outs = bass2jax._bass_exec_p.bind(
            *operands,
            out_avals=tuple(out_avals),
            in_names=tuple(all_names),
            out_names=tuple(out_names),
            lowering_input_output_aliases=(),
            sim_require_finite=True,
            sim_require_nnan=True,
            nc=nc,
        )
        return tuple(outs)

    devices = jax.devices()[:n_cores]
    mesh = Mesh(np.asarray(devices), ("core",))
    spec = PartitionSpec("core")
    in_specs = (spec,) * (n_params + len(out_names))
    out_specs = (spec,) * len(out_names)
    sharded = jax.jit(
        shard_map(_body, mesh=mesh, in_specs=in_specs, out_specs=out_specs,
                  check_rep=False),
        keep_unused=True,
    )
    sharding = NamedSharding(mesh, spec)
    concat_in = [
        jax.device_put(
            np.concatenate([np.asarray(in_maps[c][nm]) for c in range(n_cores)], axis=0),
            sharding,
        )
        for nm in in_names
    ]
    concat_zeros = [
        jax.device_put(np.zeros((n_cores * z.shape[0], *z.shape[1:]), z.dtype), sharding)
        for z in zero_outs
    ]
    args = concat_in + concat_zeros
    for _ in range(warmup):
        jax.block_until_ready(sharded(*args))
    t0 = time.perf_counter()
    outs = None
    for _ in range(iters):
        outs = sharded(*args)
    jax.block_until_ready(outs)
    t1 = time.perf_counter()
    return (t1 - t0) / iters * 1e9


def measure_hw_time(iters=30, warmup=3):
    """Per-core device execution time of the last kernel() call, in ns.

    Wall-clock per-call time through the axon tunnel carries a large fixed
    dispatch/sync overhead, so instead we compile a second NEFF whose body
    repeats the computation 4x and report the slope:
        t_device = (t(R=4) - t(R=1)) / 3
    which cancels every per-call cost that does not scale with device work.
    """
    assert _LAST_RUN is not None, "call kernel() first"
    nc, in_maps = _LAST_RUN
    C = in_maps[0]["x"].shape[1] // (H // P)
    t1 = benchmark(iters=iters, warmup=warmup, nc=nc, in_maps=in_maps)
    nc4 = _build_nc(C, repeat=4)
    t4 = benchmark(iters=iters, warmup=warmup, nc=nc4, in_maps=in_maps)
    return (t4 - t1) / 3


def kernel(x, gate_w, gate_up_w, down_w):
    from concourse.bass_utils import run_bass_kernel_spmd

    x = np.asarray(x, dtype=np.float32)
    gate_w = np.asarray(gate_w, dtype=np.float32)
    gate_up_w = np.asarray(gate_up_w, dtype=np.float32)
    down_w = np.asarray(down_w, dtype=np.float32)

    B, S, _ = x.shape
    xf = x.reshape(-1, H)
    T = xf.shape[0]

    probs, idx, tp = _route(xf, gate_w)

    tok, wgt = [], []
    for e in range(E):
        t_ids, k_ids = np.nonzero(idx == e)
        tok.append(t_ids)
        wgt.append(tp[t_ids, k_ids].astype(np.float32))
    counts = [len(t) for t in tok]
    C = max(NT, -(-max(counts) // 64) * 64)
    r = C % 512
    if 128 < r < 256:
        C += 256 - r        # keep every PSUM sub-chunk >= 256 tokens

    nc = _get_nc(C)

    in_maps = []
    for e in range(E):
        xg = np.zeros((C, H), np.float32)
        xg[: counts[e]] = xf[tok[e]]
        xh = np.ascontiguousarray(
            xg.reshape(C, H // P, P).transpose(2, 1, 0)
        ).reshape(P, (H // P) * C)
        w1h, wdh = _pack_expert_weights(gate_up_w[e], down_w[e])
        in_maps.append({"x": xh, "w1": w1h, "wd": wdh})

    res = run_bass_kernel_spmd(nc, in_maps, core_ids=list(range(E)))
    global _LAST_RUN
    _LAST_RUN = (nc, in_maps)

    out = np.zeros((T, H), np.float32)
    for e in range(E):
        if counts[e]:
            ye = res.results[e]["y"]                     # [H, C]
            out[tok[e]] += wgt[e][:, None] * ye.T[: counts[e]]

    usage = np.bincount(idx.ravel(), minlength=E).astype(np.float32)
    usage = usage / np.float32(T * TOP_K + 1e-9)
    importance = probs.mean(axis=0).astype(np.float32)
    aux = np.float32(np.sum(usage * importance) * E * LB_WEIGHT)
    aux = np.minimum(aux, np.float32(1.0))

    return out.reshape(B, S, H), aux


# revision 28
# speedup vs baseline: 1.2208x; 1.2208x over previous
"""MoE FFN layer (8 experts, top-2, SwiGLU) on 8 Trainium2 NeuronCores.

Strategy: expert parallelism. The gate (softmax + top-2 + combine weights +
aux loss) is a tiny [T,8] computation done on host as part of the
dispatch/combine (all-to-all) step. Each of the 8 cores runs ONE expert's
SwiGLU FFN over the tokens routed to it (capacity-padded so all cores run
the same SPMD program):

    hT  [2I, C] = w_gu  @ x_e.T          (K = H = 1024)
    aT  [ I, C] = silu(hT[:I]) * hT[I:]
    yT  [ H, C] = w_down @ aT            (K = I = 4096)

All tensors are pre-arranged on host into the exact SBUF layouts the
TensorEngine wants (K on partitions), so the device kernel is pure
matmul + activation with zero on-chip transposes. The weighted
scatter-add combine happens at unshard time on host.

Hardcoded problem shape: x [4, 2048, 1024], E=8, I=4096, top-2.
"""

import numpy as np

H = 1024
I = 4096
E = 8
TOP_K = 2
LB_WEIGHT = 0.01
NT = 512          # token tile (matmul moving-operand free dim)
P = 128

# matmul dtype: float32r = relaxed fp32. On TRN2 it streams at 1 PE
# cycle/row (bf16 speed) when the moving free dim is >= 256, vs 4
# cycles/row for exact fp32 — at ~2.5e-4 relative error end to end.
_MM_DTYPE_NAME = "float32r"

_NC_CACHE = {}


SUPER = 640       # max tokens per weight-streaming block (bounded by SBUF)


def _blocks_for(C):
    """Split C into weight-streaming blocks. Two goals, in order:
    1. Minimize the number of PSUM sub-chunks (each block splits into
       sub-chunks of <=512): matmul instruction count scales with sub-chunk
       count and each instruction carries ~25ns of unmodeled HW overhead.
    2. Keep every sub-chunk >= 256 (float32r runs 1 PE cycle/row only for
       moving free dim >= 256; 4 cycles/row below).
    C is a multiple of 64 and >= 512 with C % 512 not in (128, 256) —
    see the capacity rounding in kernel(). A remainder r <= 128 folds into
    a final (512+r) block (subs [256+r, 256]); r >= 256 is its own block."""
    assert C >= 512 and C % 64 == 0
    a, r = divmod(C, 512)
    assert r == 0 or r <= 128 or r >= 256, f"bad capacity {C}"
    blocks = [512] * a
    if 0 < r <= 128:
        blocks = blocks[:-1] + [512 + r]
    elif r:
        blocks.append(r)
    return blocks


def _subs_for(b):
    if b <= 512:
        return [b]
    return [b - 256, 256]


def _chunks_for(C):
    out = []
    for b in _blocks_for(C):
        out.extend(_subs_for(b))
    return out


def _emit_ffn(tc, nc, mybir, x, w1, wd, y, C):
    """Emit the per-core fused SwiGLU FFN.

    x  : DRAM [128, 8*C]   x[p, ko*C + n] = x_tok[n, ko*128 + p]
    w1 : DRAM [8192, 1024] w1[(pair*2+s)*128 + p, ko*128 + m]
                               = gate_up[s*I + pair*128 + m, ko*128 + p]
    wd : DRAM [1024, 4096] wd[m2*128 + p, k*128 + m] = down[m2*128 + m, k*128 + p]
    y  : DRAM [1024, C]    y[m2*128 + m, n] = out_tok[n, m2*128 + m]
    """
    import concourse.bass as bass  # noqa: F401

    f32 = mybir.dt.float32
    mmdt = getattr(mybir.dt, _MM_DTYPE_NAME)
    sigmoid = mybir.ActivationFunctionType.Sigmoid
    KO = H // P            # 8  k-tiles in stage 1
    NPAIR = I // P         # 32 (g,u) row-pair tiles
    K2 = I // P            # 32 k-tiles in stage 2
    M2 = H // P            # 8  output row tiles

    x3 = x.rearrange("p (ko n) -> p ko n", n=C)
    w13 = w1.rearrange("(pr s p) c -> p pr s c", s=2, p=P)
    wd3h = wd.rearrange("(m2 p) (h k c) -> p m2 h k c", p=P, h=2, c=P)
    y3 = y.rearrange("(m2 p) n -> p m2 n", p=P)

    NPRE = 2               # weight pairs prefetched across the block boundary

    with (
        tc.tile_pool(name="xp", bufs=2) as xp,
        tc.tile_pool(name="w1p", bufs=3) as w1p,
        tc.tile_pool(name="wdp", bufs=4) as wdp,
        tc.tile_pool(name="ap", bufs=1) as apool,
        tc.tile_pool(name="sp", bufs=2) as spool,
        tc.tile_pool(name="s2p", bufs=2) as spool2,
        tc.tile_pool(name="yp", bufs=2) as ypool,
        tc.tile_pool(name="ps", bufs=8, space="PSUM") as psum,
    ):
        blocks = _blocks_for(C)
        starts = [sum(blocks[:i]) for i in range(len(blocks))]

        def load_x(bi):
            xt = xp.tile([P, KO, SUPER], mmdt, tag="x")
            nc.sync.dma_start(
                xt[:, :, :blocks[bi]], x3[:, :, starts[bi]:starts[bi] + blocks[bi]]
            )
            return xt

        def load_wgu(pair):
            # one DMA brings this pair's g row-block and u row-block
            wgu = w1p.tile([P, 2, H], mmdt, tag="w1")
            nc.sync.dma_start(wgu[:], w13[:, pair, :, :])
            return wgu

        # Startup: load block 0's x in per-ko slices interleaved with the first
        # weight tiles, so pair 0's first matmul (which reads only ko=0) can
        # start after ~1.3MB of DMA instead of waiting for all ~4MB.
        xt = xp.tile([P, KO, SUPER], mmdt, tag="x")
        S0 = blocks[0]
        nc.sync.dma_start(xt[:, 0, :S0], x3[:, 0, :S0])
        wgu_pre = [load_wgu(0)]
        for ko in range(1, KO):
            nc.sync.dma_start(xt[:, ko, :S0], x3[:, ko, :S0])
            if ko == 1:
                wgu_pre.append(load_wgu(1))

        for bi, S in enumerate(blocks):
            n0 = starts[bi]
            subs = []
            o = 0
            for nt in _subs_for(S):
                subs.append((o, nt))
                o += nt

            at = apool.tile([P, K2, SUPER], mmdt, tag="a")

            for pair in range(NPAIR):
                wgu = wgu_pre[pair] if pair < NPRE else load_wgu(pair)
                for (o, nt) in subs:
                    pg = psum.tile([P, nt], f32, tag="ps")
                    pu = psum.tile([P, nt], f32, tag="ps")
                    for ko in range(KO):
                        nc.tensor.matmul(
                            pg[:],
                            wgu[:, 0, ko * P:(ko + 1) * P],
                            xt[:, ko, o:o + nt],
                            start=(ko == 0),
                            stop=(ko == KO - 1),
                        )
                    for ko in range(KO):
                        nc.tensor.matmul(
                            pu[:],
                            wgu[:, 1, ko * P:(ko + 1) * P],
                            xt[:, ko, o:o + nt],
                            start=(ko == 0),
                            stop=(ko == KO - 1),
                        )
                    sg = spool.tile([P, NT], f32, tag="s")
                    nc.scalar.activation(sg[:, :nt], pg[:], sigmoid)
                    sm = spool2.tile([P, NT], f32, tag="s2")
                    nc.vector.tensor_mul(sm[:, :nt], sg[:, :nt], pg[:])
                    nc.vector.tensor_mul(at[:, pair, o:o + nt], sm[:, :nt], pu[:])

            # Prefetch the next block's x and first weight pairs NOW, so those
            # DMAs enter the FIFO sync ring ahead of stage-2's ~10MB of
            # y-writes (else the next block's first matmuls stall ~6us).
            if bi + 1 < len(blocks):
                xt = load_x(bi + 1)
                wgu_pre = [load_wgu(p) for p in range(NPRE)]

            for m2 in range(M2):
                # wd streamed in halves for finer prefetch granularity
                wlo = wdp.tile([P, K2 // 2, P], mmdt, tag="wd")
                nc.sync.dma_start(wlo[:], wd3h[:, m2, 0])
                whi = wdp.tile([P, K2 // 2, P], mmdt, tag="wd")
                nc.sync.dma_start(whi[:], wd3h[:, m2, 1])
                ysb = ypool.tile([P, SUPER], f32, tag="y")
                for (o, nt) in subs:
                    py = psum.tile([P, nt], f32, tag="ps")
                    for k in range(K2):
                        wdt = wlo if k < K2 // 2 else whi
                        nc.tensor.matmul(
                            py[:],
                            wdt[:, k % (K2 // 2), :],
                            at[:, k, o:o + nt],
                            start=(k == 0),
                            stop=(k == K2 - 1),
                        )
                    nc.vector.tensor_copy(ysb[:, o:o + nt], py[:])
                nc.sync.dma_start(y3[:, m2, n0:n0 + S], ysb[:, :S])


def _build_nc(C, repeat=1):
    import concourse.tile as tile
    from concourse import bacc, mybir

    nc = bacc.Bacc("TRN2", target_bir_lowering=False, debug=False)
    f32 = mybir.dt.float32
    mmdt = getattr(mybir.dt, _MM_DTYPE_NAME)
    x = nc.dram_tensor("x", [P, (H // P) * C], mmdt, kind="ExternalInput").ap()
    w1 = nc.dram_tensor("w1", [2 * I, H], mmdt, kind="ExternalInput").ap()
    wd = nc.dram_tensor("wd", [H, I], mmdt, kind="ExternalInput").ap()
    y = nc.dram_tensor("y", [H, C], f32, kind="ExternalOutput").ap()
    with tile.TileContext(nc) as tc:
        for _ in range(repeat):
            _emit_ffn(tc, nc, mybir, x, w1, wd, y, C)
    nc.compile()
    return nc


def _get_nc(C):
    if C not in _NC_CACHE:
        _NC_CACHE[C] = _build_nc(C)
    return _NC_CACHE[C]


def _route(xf, gate_w):
    """Host gating: returns (probs, top2 idx, normalized top2 probs)."""
    logits = xf @ gate_w.T                                   # [T, E] f32
    m = logits.max(axis=-1, keepdims=True)
    ex = np.exp(logits - m)
    probs = ex / ex.sum(axis=-1, keepdims=True)
    order = np.argsort(-probs, axis=1, kind="stable")
    idx = order[:, :TOP_K]                                   # [T, 2]
    tp = np.take_along_axis(probs, idx, axis=1)
    tp = tp / (tp.sum(axis=-1, keepdims=True) + 1e-9)
    return probs, idx, tp


def _pack_expert_weights(gate_up_e, down_e):
    w1h = np.ascontiguousarray(
        gate_up_e.reshape(2, I // P, P, H // P, P).transpose(1, 0, 4, 3, 2)
    ).reshape(2 * I, H)
    wdh = np.ascontiguousarray(
        down_e.reshape(H // P, P, I // P, P).transpose(0, 3, 2, 1)
    ).reshape(H, I)
    return w1h, wdh


_LAST_RUN = None


def benchmark(iters=10, warmup=2, nc=None, in_maps=None):
    """Wall-clock the device execution of the last kernel() call's NEFF.

    Re-drives the same PJRT/shard_map path run_bass_kernel_spmd uses under
    axon, but with device-resident inputs and repeated pipelined calls so
    the per-call time approximates actual device execution time (max over
    the 8 cores). Returns ns per iteration.
    """
    import time
    import jax
    import numpy as np
    from jax.experimental.shard_map import shard_map
    from jax.sharding import Mesh, NamedSharding, PartitionSpec
    from concourse import bass2jax, mybir

    if nc is None:
        assert _LAST_RUN is not None, "call kernel() first"
        nc, in_maps = _LAST_RUN
    n_cores = len(in_maps)
    bass2jax.install_neuronx_cc_hook()

    partition_name = (
        nc.partition_id_tensor.name if nc.partition_id_tensor else None
    )
    in_names, out_names, out_avals, zero_outs = [], [], [], []
    for alloc in nc.m.functions[0].allocations:
        if not isinstance(alloc, bass2jax.mybir.MemoryLocationSet):
            continue
        name = alloc.memorylocations[0].name
        if alloc.kind == "ExternalInput":
            if name != partition_name:
                in_names.append(name)
        elif alloc.kind == "ExternalOutput":
            out_names.append(name)
            shape = tuple(alloc.tensor_shape)
            dtype = mybir.dt.np(alloc.dtype)
            out_avals.append(jax.core.ShapedArray(shape, dtype))
            zero_outs.append(np.zeros(shape, dtype))
    n_params = len(in_names)
    all_names = in_names + out_names
    if partition_name is not None:
        all_names = all_names + [partition_name]

    def _body(*args):
        operands = list(args)
        if partition_name is not None:
            operands.append(bass2jax.partition_id_tensor())
        outs = bass2jax._bass_exec_p.bind(
            *operands,
            out_avals=tuple(out_avals),
            in_names=tuple(all_names),
            out_names=tuple(out_names),
            lowering_input_output_aliases=(),
            sim_require_finite=True,
            sim_require_nnan=True,
            nc=nc,
        )
        return tuple(outs)

    devices = jax.devices()[:n_cores]
    mesh = Mesh(np.asarray(devices), ("core",))
    spec = PartitionSpec("core")
    in_specs = (spec,) * (n_params + len(out_names))
    out_specs = (spec,) * len(out_names)
    sharded = jax.jit(
        shard_map(_body, mesh=mesh, in_specs=in_specs, out_specs=out_specs,
                  check_rep=False),
        keep_unused=True,
    )
    sharding = NamedSharding(mesh, spec)
    concat_in = [
        jax.device_put(
            np.concatenate([np.asarray(in_maps[c][nm]) for c in range(n_cores)], axis=0),
            sharding,
        )
        for nm in in_names
    ]
    concat_zeros = [
        jax.device_put(np.zeros((n_cores * z.shape[0], *z.shape[1:]), z.dtype), sharding)
        for z in zero_outs
    ]
    args = concat_in + concat_zeros
    for _ in range(warmup):
        jax.block_until_ready(sharded(*args))
    t0 = time.perf_counter()
    outs = None
    for _ in range(iters):
        outs = sharded(*args)
    jax.block_until_ready(outs)
    t1 = time.perf_counter()
    return (t1 - t0) / iters * 1e9


def measure_hw_time(iters=30, warmup=3):
    """Per-core device execution time of the last kernel() call, in ns.

    Wall-clock per-call time through the axon tunnel carries a large fixed
    dispatch/sync overhead, so instead we compile a second NEFF whose body
    repeats the computation 4x and report the slope:
        t_device = (t(R=4) - t(R=1)) / 3
    which cancels every per-call cost that does not scale with device work.
    """
    assert _LAST_RUN is not None, "call kernel() first"
    nc, in_maps = _LAST_RUN
    C = in_maps[0]["x"].shape[1] // (H // P)
    t1 = benchmark(iters=iters, warmup=warmup, nc=nc, in_maps=in_maps)
    nc4 = _build_nc(C, repeat=4)
    t4 = benchmark(iters=iters, warmup=warmup, nc=nc4, in_maps=in_maps)
    return (t4 - t1) / 3


def kernel(x, gate_w, gate_up_w, down_w):
    from concourse.bass_utils import run_bass_kernel_spmd

    x = np.asarray(x, dtype=np.float32)
    gate_w = np.asarray(gate_w, dtype=np.float32)
    gate_up_w = np.asarray(gate_up_w, dtype=np.float32)
    down_w = np.asarray(down_w, dtype=np.float32)

    B, S, _ = x.shape
    xf = x.reshape(-1, H)
    T = xf.shape[0]

    probs, idx, tp = _route(xf, gate_w)

    tok, wgt = [], []
    for e in range(E):
        t_ids, k_ids = np.nonzero(idx == e)
        tok.append(t_ids)
        wgt.append(tp[t_ids, k_ids].astype(np.float32))
    counts = [len(t) for t in tok]
    C = max(NT, -(-max(counts) // 64) * 64)
    r = C % 512
    if 128 < r < 256:
        C += 256 - r        # keep every PSUM sub-chunk >= 256 tokens

    nc = _get_nc(C)

    in_maps = []
    for e in range(E):
        xg = np.zeros((C, H), np.float32)
        xg[: counts[e]] = xf[tok[e]]
        xh = np.ascontiguousarray(
            xg.reshape(C, H // P, P).transpose(2, 1, 0)
        ).reshape(P, (H // P) * C)
        w1h, wdh = _pack_expert_weights(gate_up_w[e], down_w[e])
        in_maps.append({"x": xh, "w1": w1h, "wd": wdh})

    res = run_bass_kernel_spmd(nc, in_maps, core_ids=list(range(E)))
    global _LAST_RUN
    _LAST_RUN = (nc, in_maps)

    out = np.zeros((T, H), np.float32)
    for e in range(E):
        if counts[e]:
            ye = res.results[e]["y"]                     # [H, C]
            out[tok[e]] += wgt[e][:, None] * ye.T[: counts[e]]

    usage = np.bincount(idx.ravel(), minlength=E).astype(np.float32)
    usage = usage / np.float32(T * TOP_K + 1e-9)
    importance = probs.mean(axis=0).astype(np.float32)
    aux = np.float32(np.sum(usage * importance) * E * LB_WEIGHT)
    aux = np.minimum(aux, np.float32(1.0))

    return out.reshape(B, S, H), aux
